# revision 6
# baseline (speedup 1.0000x reference)
"""Trainium2 Bass kernel for nn_MathematicalAttention_86139864089199.

Math (per batch element b):
    Q = x[b] @ Wq.T + bq ; K = ... ; V = ...          (reshape into 16 heads of 64)
    scores = Q K^T / 8 + structure_bias[h]
    attention = softmax(scores, axis=-1)
    context = attention @ V

Sharding: batch-parallel, one batch element per NeuronCore (B == 8 == n_cores).

On-device dataflow (per core):
    - host passes x[b].T and W.T so projections emit Q^T/K^T ([e, s] layout)
      and V ([s, e] layout) directly from the PE without extra transposes
    - scores are computed directly in transposed [k, q] orientation:
      scoresT = Kh^T(stationary) x Qh^T,  so softmax exp runs once on ACT
    - exp tiles feed the context matmul (V|ones stationary) whose extra
      "ones" column yields the softmax denominator Z[q] for free
    - 1/Z is broadcast across partitions with a K=1 PE outer product, the
      attention output is exp * (1/Z) on DVE, written to HBM as [h, k, q]
      (the host gather step transposes back to [h, q, k])
    - context^T is normalized in [e, q] layout, transposed back with the PE
      and written as [q, e]

The kernel is self-contained: shapes/sharding hardcoded below.
"""

import math
import numpy as np

B = 8
S = 1024
D = 1024
H = 16
DH = 64
P = 128

N_CORES = 8

# attention tensor device dtype: "float32" (bit-safe) or "float16" (half the
# HBM traffic + faster DVE scaling; ~5e-4 relative error on attention)
ATT_DTYPE = "float32"


def _build_nc(att_dtype_name: str):
    import concourse.bass as bass
    import concourse.bacc as bacc
    import concourse.mybir as mybir
    import concourse.tile as tile
    from concourse.masks import make_identity

    f32 = mybir.dt.float32
    att_dt = getattr(mybir.dt, att_dtype_name)

    nc = bacc.Bacc("TRN2", debug=False)

    # ---- DRAM I/O ----
    xT_d = nc.dram_tensor("xT", [D, S], f32, kind="ExternalInput")
    wT_d = {
        "q": nc.dram_tensor("wqT", [D, D], f32, kind="ExternalInput"),
        "k": nc.dram_tensor("wkT", [D, D], f32, kind="ExternalInput"),
        "v": nc.dram_tensor("wvT", [D, D], f32, kind="ExternalInput"),
    }
    bq_d = nc.dram_tensor("bq_pt", [P, D // P], f32, kind="ExternalInput")
    bk_d = nc.dram_tensor("bk_pt", [P, D // P], f32, kind="ExternalInput")
    bv_d = nc.dram_tensor("bv_row", [1, D], f32, kind="ExternalInput")
    sb_d = nc.dram_tensor("sb_bias", [P, H], f32, kind="ExternalInput")

    attnT_d = nc.dram_tensor("attnT", [H, S, S], att_dt, kind="ExternalOutput")
    ctx_d = nc.dram_tensor("ctx_out", [S, D], f32, kind="ExternalOutput")

    ND = D // P       # 8 chunks of the model/contraction dim
    NS = S // P       # 8 chunks of the sequence dim
    NH2 = 512         # matmul moving free dim

    with tile.TileContext(nc) as tc:
        with (
            tc.tile_pool(name="consts", bufs=1) as consts,
            tc.tile_pool(name="qkv", bufs=1) as qkv_pool,
        ):
            xt_pool = tc.alloc_tile_pool(name="xt", bufs=1)
            wt_pool = tc.alloc_tile_pool(name="wt", bufs=1)
            pproj = tc.alloc_tile_pool(name="pproj", bufs=3, space="PSUM")
            identity = consts.tile([P, P], f32, name="identity")
            make_identity(nc, identity)
            ones_row = consts.tile([1, NH2], f32, name="ones_row")
            nc.vector.memset(ones_row[:], 1.0)

            bq_pt = consts.tile([P, ND], f32, name="bq_pt_s")
            nc.sync.dma_start(out=bq_pt[:], in_=bq_d[:, :])
            bk_pt = consts.tile([P, ND], f32, name="bk_pt_s")
            nc.sync.dma_start(out=bk_pt[:], in_=bk_d[:, :])
            bv_row = consts.tile([1, D], f32, name="bv_row_s")
            nc.sync.dma_start(out=bv_row[:], in_=bv_d[:, :])
            sb_bias = consts.tile([P, H], f32, name="sb_bias_s")
            nc.sync.dma_start(out=sb_bias[:], in_=sb_d[:, :])

            # x^T resident in SBUF: 8 chunks [128 d, 1024 s]
            xt = []
            for d in range(ND):
                t = xt_pool.tile([P, S], f32, name=f"xt{d}", tag=f"xt{d}")
                nc.sync.dma_start(out=t[:], in_=xT_d[d * P:(d + 1) * P, :])
                xt.append(t)

            # ---- projections ----
            # Q^T / K^T: [e, s] layout, 8 partition-chunks of e
            qt = []
            kt = []
            for name, out_list, bias_pt in (("q", qt, bq_pt), ("k", kt, bk_pt)):
                w = []
                for d in range(ND):
                    t = wt_pool.tile([P, D], f32, name=f"w{name}{d}", tag=f"w{d}")
                    nc.sync.dma_start(out=t[:], in_=wT_d[name][d * P:(d + 1) * P, :])
                    w.append(t)
                for c in range(ND):
                    out_sb = qkv_pool.tile([P, S], f32, name=f"{name}t{c}",
                                           tag=f"{name}t{c}")
                    for sh in range(2):
                        ps = pproj.tile([P, NH2], f32, name="ps_proj", tag="pp")
                        for d in range(ND):
                            nc.tensor.matmul(
                                ps[:],
                                w[d][:, c * P:(c + 1) * P],
                                xt[d][:, sh * NH2:(sh + 1) * NH2],
                                start=(d == 0),
                                stop=(d == ND - 1),
                            )
                        # eviction with fused per-partition bias add
                        nc.scalar.add(out_sb[:, sh * NH2:(sh + 1) * NH2], ps[:],
                                      bias_pt[:, c:c + 1])
                    out_list.append(out_sb)

            # V: natural [s, e] layout, extended with a ones column per head:
            # vx[s_tile] has shape [128, H, DH+1]
            vx = []
            for c in range(NS):
                t = qkv_pool.tile([P, H, DH + 1], att_dt, name=f"vx{c}",
                                  tag=f"vx{c}")
                nc.vector.memset(t[:, :, DH:DH + 1], 1.0)
                vx.append(t)
            w = []
            for d in range(ND):
                t = wt_pool.tile([P, D], f32, name=f"wv{d}", tag=f"w{d}")
                nc.sync.dma_start(out=t[:], in_=wT_d["v"][d * P:(d + 1) * P, :])
                w.append(t)
            for c in range(NS):
                for eh in range(2):
                    ps = pproj.tile([P, NH2], f32, name="ps_projv", tag="pp")
                    for d in range(ND):
                        nc.tensor.matmul(
                            ps[:],
                            xt[d][:, c * P:(c + 1) * P],
                            w[d][:, eh * NH2:(eh + 1) * NH2],
                            start=(d == 0),
                            stop=False,
                        )
                    # + bv[e] via a K=1 rank-1 update with a ones column
                    nc.tensor.matmul(
                        ps[:],
                        ones_row[0:1, 0:P],
                        bv_row[0:1, eh * NH2:(eh + 1) * NH2],
                        start=False,
                        stop=True,
                    )
                    nc.vector.tensor_copy(
                        vx[c][:, eh * (H // 2):(eh + 1) * (H // 2), 0:DH], ps[:])

            # phase-1-only pools: release so attention pools can reuse the space
            pproj.release()
            wt_pool.release()
            xt_pool.release()

            # ---- attention, head by head ----
            with (
                tc.tile_pool(name="ps_s", bufs=2, space="PSUM") as ps_s_pool,
                tc.tile_pool(name="ps_ctx", bufs=2, space="PSUM") as ps_ctx_pool,
                tc.tile_pool(name="ps_aux", bufs=1, space="PSUM") as ps_aux_pool,
                tc.tile_pool(name="att_sb", bufs=2) as att_pool,
            ):
                for h in range(H):
                    qhT = qt[h // 2][(h % 2) * DH:(h % 2) * DH + DH, :]
                    khT = kt[h // 2][(h % 2) * DH:(h % 2) * DH + DH, :]

                    # scores^T [k, q] -> exp tiles (one ACT pass, scale=1/8,
                    # bias = structure_bias[h] - 3 fused)
                    ex = []
                    for kc in range(NS):
                        ps = ps_s_pool.tile([P, S], f32, name="ps_s", tag="ps")
                        for qh2 in range(2):
                            nc.tensor.matmul(
                                ps[:, qh2 * NH2:(qh2 + 1) * NH2],
                                khT[:, kc * P:(kc + 1) * P],
                                qhT[:, qh2 * NH2:(qh2 + 1) * NH2],
                                start=True,
                                stop=True,
                            )
                        e_t = att_pool.tile([P, S], att_dt, name=f"ex{kc}",
                                            tag=f"ex{kc}", bufs=2)
                        nc.scalar.activation(
                            e_t[:], ps[:],
                            mybir.ActivationFunctionType.Exp,
                            bias=sb_bias[:, h:h + 1],
                            scale=0.125,
                        )
                        ex.append(e_t)

                    # context^T accumulation, with ones column -> Z in row 64
                    ps_ctx = []
                    for qh2 in range(2):
                        pc = ps_ctx_pool.tile([DH + 1, NH2], f32,
                                              name=f"ps_ctx{qh2}", tag="ctxT")
                        ps_ctx.append(pc)
                    for kc in range(NS):
                        for qh2 in range(2):
                            nc.tensor.matmul(
                                ps_ctx[qh2][:],
                                vx[kc][:, h, :],
                                ex[kc][:, qh2 * NH2:(qh2 + 1) * NH2],
                                start=(kc == 0),
                                stop=(kc == NS - 1),
                            )

                    # Z row -> SBUF
                    z_row = att_pool.tile([1, S], f32, name="z_row", tag="zrow",
                                          bufs=2)
                    for qh2 in range(2):
                        nc.vector.tensor_copy(
                            z_row[:, qh2 * NH2:(qh2 + 1) * NH2],
                            ps_ctx[qh2][DH:DH + 1, :])

                    # broadcast Z across partitions (K=1 outer product), recip
                    bc_ps = ps_aux_pool.tile([P, S], f32, name="bc_ps", tag="aux")
                    for qh2 in range(2):
                        nc.tensor.matmul(
                            bc_ps[:, qh2 * NH2:(qh2 + 1) * NH2],
                            ones_row[0:1, 0:P],
                            z_row[0:1, qh2 * NH2:(qh2 + 1) * NH2],
                            start=True,
                            stop=True,
                        )
                    rzb = att_pool.tile([P, S], f32, name="rzb", tag="rzb", bufs=2)
                    nc.vector.reciprocal(rzb[:], bc_ps[:])
                    if att_dt != f32:
                        rzb_c = att_pool.tile([P, S], att_dt, name="rzb_c",
                                              tag="rzb_c", bufs=2)
                        nc.vector.tensor_copy(rzb_c[:], rzb[:])
                    else:
                        rzb_c = rzb

                    # attention output tiles: exp * (1/Z), written as [k, q]
                    for kc in range(NS):
                        a_t = att_pool.tile([P, S], att_dt, name="a_t", tag="a_t",
                                            bufs=3)
                        nc.vector.tensor_mul(a_t[:], ex[kc][:], rzb_c[:])
                        nc.sync.dma_start(
                            out=attnT_d[h, kc * P:(kc + 1) * P, :], in_=a_t[:])

                    # normalize context^T in [e, q] layout, transpose to [q, e]
                    ctxTs = att_pool.tile([DH, S], f32, name="ctxTs", tag="ctxTs",
                                          bufs=2)
                    for qh2 in range(2):
                        nc.vector.tensor_mul(
                            ctxTs[:, qh2 * NH2:(qh2 + 1) * NH2],
                            ps_ctx[qh2][0:DH, :],
                            rzb[0:DH, qh2 * NH2:(qh2 + 1) * NH2])

                    ctxn_ps = ps_aux_pool.tile([P, NS * DH], f32, name="ctxn_ps",
                                               tag="aux")
                    for c in range(NS):
                        nc.tensor.transpose(
                            ctxn_ps[:, c * DH:(c + 1) * DH],
                            ctxTs[:, c * P:(c + 1) * P],
                            identity[0:DH, 0:DH],
                        )
                    ctx_sb = att_pool.tile([P, NS, DH], f32, name="ctx_sb",
                                           tag="ctx_sb", bufs=2)
                    nc.scalar.copy(ctx_sb[:], ctxn_ps[:])
                    nc.sync.dma_start(
                        out=ctx_d[:, h * DH:(h + 1) * DH].rearrange(
                            "(c p) e -> p c e", p=P),
                        in_=ctx_sb[:],
                    )
    nc.compile()
    return nc


_NC_CACHE = {}


def _get_nc():
    if ATT_DTYPE not in _NC_CACHE:
        _NC_CACHE[ATT_DTYPE] = _build_nc(ATT_DTYPE)
    return _NC_CACHE[ATT_DTYPE]


def _prep_inputs(x, Wq, bq, Wk, bk, Wv, bv, structure_bias):
    """Build the per-core input maps (host-side sharding/layout prep)."""
    wqT = np.ascontiguousarray(np.asarray(Wq, np.float32).T)
    wkT = np.ascontiguousarray(np.asarray(Wk, np.float32).T)
    wvT = np.ascontiguousarray(np.asarray(Wv, np.float32).T)
    bq_pt = np.ascontiguousarray(np.asarray(bq, np.float32).reshape(D // P, P).T)
    bk_pt = np.ascontiguousarray(np.asarray(bk, np.float32).reshape(D // P, P).T)
    bv_row = np.asarray(bv, np.float32).reshape(1, D).copy()
    sb = np.asarray(structure_bias, np.float32).reshape(1, H) - 3.0
    sb_bias = np.ascontiguousarray(np.broadcast_to(sb, (P, H)))

    in_maps = []
    for b in range(N_CORES):
        xT = np.ascontiguousarray(np.asarray(x[b], np.float32).T)
        in_maps.append({
            "xT": xT,
            "wqT": wqT, "wkT": wkT, "wvT": wvT,
            "bq_pt": bq_pt, "bk_pt": bk_pt, "bv_row": bv_row,
            "sb_bias": sb_bias,
        })
    return in_maps


def _gather(results):
    context = np.empty((B, S, D), np.float32)
    attention = np.empty((B, H, S, S), np.float32)
    for b in range(B):
        context[b] = results[b]["ctx_out"]
        at = results[b]["attnT"]  # [H, k, q]
        attention[b] = at.transpose(0, 2, 1).astype(np.float32)
    return context, attention


def kernel(x, Wq, bq, Wk, bk, Wv, bv, structure_bias, _trace=False):
    from concourse.bass_utils import run_bass_kernel_spmd

    nc = _get_nc()
    in_maps = _prep_inputs(x, Wq, bq, Wk, bk, Wv, bv, structure_bias)
    res = run_bass_kernel_spmd(nc, in_maps, core_ids=list(range(N_CORES)),
                               trace=_trace)
    out = _gather(res.results)
    if _trace:
        return out, res
    return out


# revision 7
# speedup vs baseline: 2.9718x; 2.9718x over previous
"""Trainium2 Bass kernel for nn_MathematicalAttention_86139864089199.

Math (per batch element b):
    Q = x[b] @ Wq.T + bq ; K = ... ; V = ...          (reshape into 16 heads of 64)
    scores = Q K^T / 8 + structure_bias[h]
    attention = softmax(scores, axis=-1)
    context = attention @ V

Sharding: batch-parallel, one batch element per NeuronCore (B == 8 == n_cores).

On-device dataflow (per core):
    - host passes x[b].T and W.T (fp16) so projections emit Q^T/K^T ([e, s]
      layout) and V ([s, e] layout) directly from the PE without transposes.
      fp16 matmul operands: PE streams fp32 at 1/4 rate, 16-bit at full rate.
    - scores are computed directly in transposed [k, q] orientation:
      scoresT = Kh^T(stationary) x Qh^T, so softmax exp runs once on ACT
      (scale=1/8 and per-head bias fused into the activation)
    - exp tiles feed the context matmul (V|ones stationary) whose extra
      "ones" column yields the softmax denominator Z[q] for free
    - 1/Z is broadcast across partitions with a K=1 PE outer product +
      reciprocal_approx_fast; attention output is exp * (1/Z) on DVE,
      written to HBM as [h, k, q] fp16 (host gather transposes to [h, q, k])
    - context^T is normalized in [e, q] layout, transposed back with the PE
      and written as [q, e] fp32

The kernel is self-contained: shapes/sharding hardcoded below.
"""

import math
import numpy as np

B = 8
S = 1024
D = 1024
H = 16
DH = 64
P = 128

N_CORES = 8


def _build_nc():
    import concourse.bass as bass
    import concourse.bacc as bacc
    import concourse.mybir as mybir
    import concourse.tile as tile
    from concourse.masks import make_identity

    f32 = mybir.dt.float32
    f16 = mybir.dt.float16

    nc = bacc.Bacc("TRN2", debug=False)

    # ---- DRAM I/O ----
    xT_d = nc.dram_tensor("xT", [D, S], f16, kind="ExternalInput")
    wT_d = {
        "q": nc.dram_tensor("wqT", [D, D], f16, kind="ExternalInput"),
        "k": nc.dram_tensor("wkT", [D, D], f16, kind="ExternalInput"),
        "v": nc.dram_tensor("wvT", [D, D], f16, kind="ExternalInput"),
    }
    bq_d = nc.dram_tensor("bq_pt", [P, D // P], f32, kind="ExternalInput")
    bk_d = nc.dram_tensor("bk_pt", [P, D // P], f32, kind="ExternalInput")
    bv_d = nc.dram_tensor("bv_row", [1, D], f16, kind="ExternalInput")
    sb_d = nc.dram_tensor("sb_bias", [P, H], f32, kind="ExternalInput")

    attnT_d = nc.dram_tensor("attnT", [H, S, S], f16, kind="ExternalOutput")
    ctx_d = nc.dram_tensor("ctx_out", [S, D], f32, kind="ExternalOutput")

    ND = D // P       # 8 chunks of the model/contraction dim
    NS = S // P       # 8 chunks of the sequence dim
    NH2 = 512         # matmul moving free dim

    with tile.TileContext(nc) as tc:
        with (
            tc.tile_pool(name="consts", bufs=1) as consts,
            tc.tile_pool(name="qkv", bufs=1) as qkv_pool,
        ):
            xt_pool = tc.alloc_tile_pool(name="xt", bufs=1)
            wt_pool = tc.alloc_tile_pool(name="wt", bufs=1)
            pproj = tc.alloc_tile_pool(name="pproj", bufs=3, space="PSUM")

            identity = consts.tile([P, P], f32, name="identity")
            make_identity(nc, identity)
            ones16 = consts.tile([1, P], f16, name="ones16")
            nc.vector.memset(ones16[:], 1.0)
            ones32 = consts.tile([1, P], f32, name="ones32")
            nc.vector.memset(ones32[:], 1.0)

            bq_pt = consts.tile([P, ND], f32, name="bq_pt_s")
            nc.sync.dma_start(out=bq_pt[:], in_=bq_d[:, :])
            bk_pt = consts.tile([P, ND], f32, name="bk_pt_s")
            nc.sync.dma_start(out=bk_pt[:], in_=bk_d[:, :])
            bv_row = consts.tile([1, D], f16, name="bv_row_s")
            nc.sync.dma_start(out=bv_row[:], in_=bv_d[:, :])
            sb_bias = consts.tile([P, H], f32, name="sb_bias_s")
            nc.sync.dma_start(out=sb_bias[:], in_=sb_d[:, :])

            # x^T resident in SBUF: 8 chunks [128 d, 1024 s]
            xt = []
            for d in range(ND):
                t = xt_pool.tile([P, S], f16, name=f"xt{d}", tag=f"xt{d}")
                nc.sync.dma_start(out=t[:], in_=xT_d[d * P:(d + 1) * P, :])
                xt.append(t)

            # ---- projections ----
            # Q^T / K^T: [e, s] layout fp16, 8 partition-chunks of e
            qt = []
            kt = []
            for name, out_list, bias_pt in (("q", qt, bq_pt), ("k", kt, bk_pt)):
                w = []
                for d in range(ND):
                    t = wt_pool.tile([P, D], f16, name=f"w{name}{d}", tag=f"w{d}")
                    nc.sync.dma_start(out=t[:], in_=wT_d[name][d * P:(d + 1) * P, :])
                    w.append(t)
                for c in range(ND):
                    out_sb = qkv_pool.tile([P, S], f16, name=f"{name}t{c}",
                                           tag=f"{name}t{c}")
                    for sh in range(2):
                        ps = pproj.tile([P, NH2], f32, name="ps_proj", tag="pp")
                        for d in range(ND):
                            nc.tensor.matmul(
                                ps[:],
                                w[d][:, c * P:(c + 1) * P],
                                xt[d][:, sh * NH2:(sh + 1) * NH2],
                                start=(d == 0),
                                stop=(d == ND - 1),
                            )
                        # eviction with fused per-partition bias add
                        nc.scalar.add(out_sb[:, sh * NH2:(sh + 1) * NH2], ps[:],
                                      bias_pt[:, c:c + 1])
                    out_list.append(out_sb)

            # V: natural [s, e] layout, extended with a ones column per head:
            # vx[s_tile] has shape [128, H, DH+1]
            vx = []
            for c in range(NS):
                t = qkv_pool.tile([P, H, DH + 1], f16, name=f"vx{c}",
                                  tag=f"vx{c}")
                nc.vector.memset(t[:, :, DH:DH + 1], 1.0)
                vx.append(t)
            w = []
            for d in range(ND):
                t = wt_pool.tile([P, D], f16, name=f"wv{d}", tag=f"w{d}")
                nc.sync.dma_start(out=t[:], in_=wT_d["v"][d * P:(d + 1) * P, :])
                w.append(t)
            for c in range(NS):
                for eh in range(2):
                    ps = pproj.tile([P, NH2], f32, name="ps_projv", tag="pp")
                    for d in range(ND):
                        nc.tensor.matmul(
                            ps[:],
                            xt[d][:, c * P:(c + 1) * P],
                            w[d][:, eh * NH2:(eh + 1) * NH2],
                            start=(d == 0),
                            stop=False,
                        )
                    # + bv[e] via a K=1 rank-1 update with a ones column
                    nc.tensor.matmul(
                        ps[:],
                        ones16[0:1, :],
                        bv_row[0:1, eh * NH2:(eh + 1) * NH2],
                        start=False,
                        stop=True,
                    )
                    nc.vector.tensor_copy(
                        vx[c][:, eh * (H // 2):(eh + 1) * (H // 2), 0:DH], ps[:])

            # phase-1-only pools: release so attention pools can reuse the space
            pproj.release()
            wt_pool.release()
            xt_pool.release()

            # ---- attention, head by head ----
            with (
                tc.tile_pool(name="ps_s", bufs=2, space="PSUM") as ps_s_pool,
                tc.tile_pool(name="ps_ctx", bufs=2, space="PSUM") as ps_ctx_pool,
                tc.tile_pool(name="ps_aux", bufs=1, space="PSUM") as ps_aux_pool,
                tc.tile_pool(name="att_sb", bufs=2) as att_pool,
            ):
                for h in range(H):
                    qhT = qt[h // 2][(h % 2) * DH:(h % 2) * DH + DH, :]
                    khT = kt[h // 2][(h % 2) * DH:(h % 2) * DH + DH, :]

                    # scores^T [k, q] -> exp tiles (one ACT pass, scale=1/8,
                    # bias = structure_bias[h] - 3 fused)
                    ex = []
                    for kc in range(NS):
                        ps = ps_s_pool.tile([P, S], f32, name="ps_s", tag="ps")
                        for qh2 in range(2):
                            nc.tensor.matmul(
                                ps[:, qh2 * NH2:(qh2 + 1) * NH2],
                                khT[:, kc * P:(kc + 1) * P],
                                qhT[:, qh2 * NH2:(qh2 + 1) * NH2],
                                start=True,
                                stop=True,
                            )
                        e_t = att_pool.tile([P, S], f16, name=f"ex{kc}",
                                            tag=f"ex{kc}", bufs=2)
                        nc.scalar.activation(
                            e_t[:], ps[:],
                            mybir.ActivationFunctionType.Exp,
                            bias=sb_bias[:, h:h + 1],
                            scale=0.125,
                        )
                        ex.append(e_t)

                    # context^T accumulation, with ones column -> Z in row 64
                    ps_ctx = []
                    for qh2 in range(2):
                        pc = ps_ctx_pool.tile([DH + 1, NH2], f32,
                                              name=f"ps_ctx{qh2}", tag="ctxT")
                        ps_ctx.append(pc)
                    for kc in range(NS):
                        for qh2 in range(2):
                            nc.tensor.matmul(
                                ps_ctx[qh2][:],
                                vx[kc][:, h, :],
                                ex[kc][:, qh2 * NH2:(qh2 + 1) * NH2],
                                start=(kc == 0),
                                stop=(kc == NS - 1),
                            )

                    # Z row -> SBUF
                    z_row = att_pool.tile([1, S], f32, name="z_row", tag="zrow",
                                          bufs=2)
                    for qh2 in range(2):
                        nc.vector.tensor_copy(
                            z_row[:, qh2 * NH2:(qh2 + 1) * NH2],
                            ps_ctx[qh2][DH:DH + 1, :])

                    # broadcast Z across partitions (K=1 outer product), recip
                    bc_ps = ps_aux_pool.tile([P, S], f32, name="bc_ps", tag="aux")
                    for qh2 in range(2):
                        nc.tensor.matmul(
                            bc_ps[:, qh2 * NH2:(qh2 + 1) * NH2],
                            ones32[0:1, :],
                            z_row[0:1, qh2 * NH2:(qh2 + 1) * NH2],
                            start=True,
                            stop=True,
                        )
                    rzb = att_pool.tile([P, S], f32, name="rzb", tag="rzb", bufs=2)
                    nc.vector.reciprocal_approx_fast(out=rzb[:], in_=bc_ps[:])
                    rzb_c = att_pool.tile([P, S], f16, name="rzb_c",
                                          tag="rzb_c", bufs=2)
                    nc.vector.tensor_copy(rzb_c[:], rzb[:])

                    # attention output tiles: exp * (1/Z), written as [k, q]
                    for kc in range(NS):
                        a_t = att_pool.tile([P, S], f16, name="a_t", tag="a_t",
                                            bufs=3)
                        nc.vector.tensor_mul(a_t[:], ex[kc][:], rzb_c[:])
                        nc.sync.dma_start(
                            out=attnT_d[h, kc * P:(kc + 1) * P, :], in_=a_t[:])

                    # normalize context^T in [e, q] layout, transpose to [q, e]
                    ctxTs = att_pool.tile([DH, S], f32, name="ctxTs", tag="ctxTs",
                                          bufs=2)
                    for qh2 in range(2):
                        nc.vector.tensor_mul(
                            ctxTs[:, qh2 * NH2:(qh2 + 1) * NH2],
                            ps_ctx[qh2][0:DH, :],
                            rzb[0:DH, qh2 * NH2:(qh2 + 1) * NH2])

                    ctxn_ps = ps_aux_pool.tile([P, NS * DH], f32, name="ctxn_ps",
                                               tag="aux")
                    for c in range(NS):
                        nc.tensor.transpose(
                            ctxn_ps[:, c * DH:(c + 1) * DH],
                            ctxTs[:, c * P:(c + 1) * P],
                            identity[0:DH, 0:DH],
                        )
                    ctx_sb = att_pool.tile([P, NS, DH], f32, name="ctx_sb",
                                           tag="ctx_sb", bufs=2)
                    nc.scalar.copy(ctx_sb[:], ctxn_ps[:])
                    nc.sync.dma_start(
                        out=ctx_d[:, h * DH:(h + 1) * DH].rearrange(
                            "(c p) e -> p c e", p=P),
                        in_=ctx_sb[:],
                    )
    nc.compile()
    return nc


_NC_CACHE = {}


def _get_nc():
    if "nc" not in _NC_CACHE:
        _NC_CACHE["nc"] = _build_nc()
    return _NC_CACHE["nc"]


def _prep_inputs(x, Wq, bq, Wk, bk, Wv, bv, structure_bias):
    """Build the per-core input maps (host-side sharding/layout prep)."""
    wqT = np.ascontiguousarray(np.asarray(Wq, np.float32).T).astype(np.float16)
    wkT = np.ascontiguousarray(np.asarray(Wk, np.float32).T).astype(np.float16)
    wvT = np.ascontiguousarray(np.asarray(Wv, np.float32).T).astype(np.float16)
    bq_pt = np.ascontiguousarray(np.asarray(bq, np.float32).reshape(D // P, P).T)
    bk_pt = np.ascontiguousarray(np.asarray(bk, np.float32).reshape(D // P, P).T)
    bv_row = np.asarray(bv, np.float32).reshape(1, D).astype(np.float16)
    sb = np.asarray(structure_bias, np.float32).reshape(1, H) - 3.0
    sb_bias = np.ascontiguousarray(np.broadcast_to(sb, (P, H)))

    in_maps = []
    for b in range(N_CORES):
        xT = np.ascontiguousarray(np.asarray(x[b], np.float32).T).astype(np.float16)
        in_maps.append({
            "xT": xT,
            "wqT": wqT, "wkT": wkT, "wvT": wvT,
            "bq_pt": bq_pt, "bk_pt": bk_pt, "bv_row": bv_row,
            "sb_bias": sb_bias,
        })
    return in_maps


def _gather(results):
    context = np.empty((B, S, D), np.float32)
    attention = np.empty((B, H, S, S), np.float32)
    for b in range(B):
        context[b] = results[b]["ctx_out"]
        at = results[b]["attnT"]  # [H, k, q] fp16
        attention[b] = at.transpose(0, 2, 1).astype(np.float32)
    return context, attention


def kernel(x, Wq, bq, Wk, bk, Wv, bv, structure_bias, _trace=False):
    from concourse.bass_utils import run_bass_kernel_spmd

    nc = _get_nc()
    in_maps = _prep_inputs(x, Wq, bq, Wk, bk, Wv, bv, structure_bias)
    res = run_bass_kernel_spmd(nc, in_maps, core_ids=list(range(N_CORES)),
                               trace=_trace)
    out = _gather(res.results)
    if _trace:
        return out, res
    return out
